# revision 11
# baseline (speedup 1.0000x reference)
"""Trainium2 Bass kernel for nn_CIFARDiffusionLayer.

The reference applies, per channel c, three ADI steps; each step is an
x-sweep (constant-coefficient tridiagonal solve along W), a y-sweep
(same along H), and a multiply by diag(channel_coupling)[c].  Every
sweep is a fixed linear map: solving T x = d with the reference's exact
Thomas recurrence is x = T^{-1} d, and T^{-1} is a dense 256x256 matrix
that depends only on (channel, step, direction).  X-sweeps act on U by
right-multiplication and y-sweeps by left-multiplication, so they all
commute across steps and the whole layer collapses to

    out[b, c] = A_c @ u[b, c] @ B_c
    A_c = s_c^3 * My(c,2) @ My(c,1) @ My(c,0)      (s_c = coupling diag)
    B_c = Mx(c,0)^T @ Mx(c,1)^T @ Mx(c,2)^T

with the tiny 256x256 matrices computed on the host in float64 from the
reference's exact recurrences (including its eps quirks).  The device
work is two 256x256x256 matmuls per (batch, channel) slab, run as
fp32r (full-rate) TensorE matmuls with the data slab as the stationary
operand so each matmul also transposes the slab back and forth.

Sharding: data parallelism over (batch, channel) slabs: 384 slabs are
dealt to 8 cores as 48 generic slabs each (32 of one channel + 16 of
another, per the ASSIGN table), so each core loads only the 2 matrix
pairs it needs (1.0MB instead of 1.5MB of constants) while the NEFF
stays identical across cores.
"""

import sys

if "/opt/trn_rl_repo" not in sys.path:
    sys.path.insert(0, "/opt/trn_rl_repo")

import numpy as np

DT = 0.05
DX = 1.0
NUM_STEPS = 3
EPS = 1e-6
MAX_COEFF = 1.0

N_CORES = 8
B, C, S = 128, 3, 256
B_LOC = B // N_CORES
N_SLAB = 48          # (batch, channel) slabs per core
N_GROUP = N_SLAB // 3
# Per core: ((channel of the 32-slab block, batch start), (channel of the
# 16-slab block, batch start)).  Covers each (b, c) exactly once:
# c0 = 4x32, c1 = 2x32 + 4x16, c2 = 2x32 + 4x16.
ASSIGN = [
    ((0, 0), (1, 64)),
    ((0, 32), (1, 80)),
    ((0, 64), (1, 96)),
    ((0, 96), (1, 112)),
    ((1, 0), (2, 64)),
    ((1, 32), (2, 80)),
    ((2, 0), (2, 96)),
    ((2, 32), (2, 112)),
]


def _core_slab_indices(k):
    (c32, b32), (c16, b16) = ASSIGN[k]
    b_idx = list(range(b32, b32 + 32)) + list(range(b16, b16 + 16))
    c_idx = [c32] * 32 + [c16] * 16
    return b_idx, c_idx


def _thomas_inv(r: float, n: int = S, eps: float = EPS) -> np.ndarray:
    """T^{-1} for the reference's constant-coefficient Thomas solve.

    Mirrors reference._thomas_const exactly (b[0]+eps on the first
    denominator, clamp(min=eps) on interior denominators), evaluated in
    float64 on the identity RHS so columns are T^{-1} e_j.
    """
    a = -r
    b = np.full(n, 1.0 + 2.0 * r, dtype=np.float64)
    b[0] = b[-1] = 1.0 + r
    denom = np.empty(n, dtype=np.float64)
    cp = np.empty(n, dtype=np.float64)
    denom[0] = b[0] + eps
    cp[0] = a / denom[0]
    for i in range(1, n):
        denom[i] = max(b[i] - a * cp[i - 1], eps)
        cp[i] = a / denom[i]
    dp = np.zeros((n, n), dtype=np.float64)
    eye = np.eye(n, dtype=np.float64)
    dp[0] = eye[0] / denom[0]
    for i in range(1, n):
        dp[i] = (eye[i] - a * dp[i - 1]) / denom[i]
    x = np.zeros((n, n), dtype=np.float64)
    x[n - 1] = dp[n - 1]
    for i in range(n - 2, -1, -1):
        x[i] = dp[i] - cp[i] * x[i + 1]
    return x


def _host_mats(alpha_base, beta_base, alpha_spatial, beta_spatial, channel_coupling):
    """mats[c, 0] = A_c^T, mats[c, 1] = B_c, as float32 [C, 2, S, S]."""
    diag = np.diagonal(np.asarray(channel_coupling)).astype(np.float64)
    mats = np.empty((C, 2, S, S), dtype=np.float32)
    for c in range(C):
        am = float(np.mean(np.asarray(alpha_spatial[c], dtype=np.float64)))
        bm = float(np.mean(np.asarray(beta_spatial[c], dtype=np.float64)))
        a_c = np.eye(S, dtype=np.float64)
        b_c = np.eye(S, dtype=np.float64)
        for step in range(NUM_STEPS):
            t = step * DT
            alpha_t = min(max(float(alpha_base[c]) + am * t, EPS), MAX_COEFF)
            beta_t = min(max(float(beta_base[c]) + bm * t, EPS), MAX_COEFF)
            r_a = alpha_t * (DT / 2.0) / DX**2
            r_b = beta_t * (DT / 2.0) / DX**2
            a_c = _thomas_inv(r_b) @ a_c
            b_c = b_c @ _thomas_inv(r_a).T
        mats[c, 0] = (diag[c] ** 3 * a_c).T.astype(np.float32)
        mats[c, 1] = b_c.astype(np.float32)
    return mats


def build_module(repeat: int = 1):
    """Per-core Bass module: out[b,c] = A_c @ u[b,c] @ B_c for 16 slabs x 3 ch.

    repeat > 1 wraps the batch loop in a hardware For_i that re-runs the
    whole kernel body; only used by the timing harness (wall-clock slope
    between two repeat counts isolates the per-iteration device time).
    """
    import concourse.bacc as bacc
    import concourse.tile as tile
    from concourse import mybir

    f32, f32r = mybir.dt.float32, mybir.dt.float32r
    nc = bacc.Bacc(
        "TRN2",
        target_bir_lowering=False,
        debug=False,
        enable_asserts=False,
        num_devices=N_CORES,
    )
    u_d = nc.dram_tensor("u", [N_SLAB, S, S], f32r, kind="ExternalInput")
    m_d = nc.dram_tensor("mats", [2, 2, S, S], f32r, kind="ExternalInput")
    o_d = nc.dram_tensor("out", [N_SLAB, S, S], f32, kind="ExternalOutput")

    with tile.TileContext(nc) as tc:
        with (
            tc.tile_pool(name="consts", bufs=1) as cpool,
            tc.tile_pool(name="ld", bufs=5) as ldpool,
            tc.tile_pool(name="vt", bufs=3) as vtpool,
            tc.tile_pool(name="zs", bufs=4) as zspool,
            tc.tile_pool(name="pv", bufs=2, space="PSUM") as pvpool,
            tc.tile_pool(name="pz", bufs=2, space="PSUM") as pzpool,
        ):
            # Matrix pair q in {0,1}; one [128, 512] tile per (pair, side):
            # [:, 0:256] = k-tile rows 0..127, [:, 256:512] = rows 128..255.
            a_t, b_t = [], []
            for q in range(2):
                at = cpool.tile([128, 512], f32r, tag=f"a{q}")
                nc.sync.dma_start(at[:], m_d[q, 0].rearrange("(k p) w -> p k w", p=128))
                a_t.append(at)
                bt = cpool.tile([128, 512], f32r, tag=f"b{q}")
                nc.sync.dma_start(bt[:], m_d[q, 1].rearrange("(k p) w -> p k w", p=128))
                b_t.append(bt)

            def batch_loop():
                for g in range(N_GROUP):
                    _emit_group(g)

            def _emit_group(g):
                # Load 3 slabs: free layout j*512 + k*256 + w, partition = h%128.
                # Per-slab DMAs keep the SP queue from head-of-line blocking.
                ld = ldpool.tile([128, 3 * 512], f32r)
                for j in range(3):
                    nc.sync.dma_start(
                        ld[:, j * 512 : (j + 1) * 512],
                        u_d[3 * g + j].rearrange("(k p) w -> p k w", p=128),
                    )
                zs = zspool.tile([128, 3 * 512], f32)
                for j in range(3):
                    slab = 3 * g + j
                    q = 0 if slab < 32 else 1
                    base = j * 512
                    # MM1: V^T[w, h'] = sum_h U[h, w] * A^T[h, h']  (data stationary)
                    pv = pvpool.tile([128, 512], f32)
                    for mi in range(2):
                        for k in range(2):
                            nc.tensor.matmul(
                                pv[:, mi * 256 : (mi + 1) * 256],
                                ld[:, base + k * 256 + mi * 128 : base + k * 256 + mi * 128 + 128],
                                a_t[q][:, k * 256 : (k + 1) * 256],
                                start=(k == 0),
                                stop=(k == 1),
                            )
                    vt = vtpool.tile([128, 512], f32r)
                    nc.vector.tensor_copy(vt[:], pv[:])
                    # MM2: Z[h', w'] = sum_w V^T[w, h'] * B[w, w']
                    pz = pzpool.tile([128, 512], f32)
                    for mi in range(2):
                        for k in range(2):
                            nc.tensor.matmul(
                                pz[:, mi * 256 : (mi + 1) * 256],
                                vt[:, k * 256 + mi * 128 : k * 256 + mi * 128 + 128],
                                b_t[q][:, k * 256 : (k + 1) * 256],
                                start=(k == 0),
                                stop=(k == 1),
                            )
                    nc.scalar.copy(zs[:, base : base + 512], pz[:])
                # Out-DMA on the ACT HWDGE ring: keeps the SP queue free for
                # input loads (out-DMAs wait on compute; SP head-of-line
                # blocking would stall the next group's loads behind them).
                nc.scalar.dma_start(
                    o_d[3 * g : 3 * g + 3].rearrange("s (k p) w -> p s k w", p=128),
                    zs[:],
                )

            if repeat == 1:
                batch_loop()
            else:
                # staggered_reset avoids the ~3us all-engine barrier at the
                # loop back-edge, so the slope measurement better matches the
                # barrier-free single-shot kernel.
                with tc.For_i(0, repeat, 1, staggered_reset=True):
                    batch_loop()
    nc.compile()
    return nc


_CACHE = {}


def _axon_runner():
    """Build (once) a jitted 8-way sharded executor for the axon/PJRT path.

    Mirrors concourse.bass2jax.run_bass_via_pjrt but keeps the compiled
    executable alive so repeat kernel() calls skip retracing + NEFF
    recompilation.
    """
    if "runner" in _CACHE:
        return _CACHE["runner"]
    import jax
    from jax.experimental.shard_map import shard_map
    from jax.sharding import Mesh, NamedSharding, PartitionSpec

    from concourse import bass2jax, mybir

    nc = build_module()
    bass2jax.install_neuronx_cc_hook()
    partition_name = nc.partition_id_tensor.name if nc.partition_id_tensor else None
    in_names, out_names, out_avals = [], [], []
    for alloc in nc.m.functions[0].allocations:
        if not isinstance(alloc, mybir.MemoryLocationSet):
            continue
        name = alloc.memorylocations[0].name
        if alloc.kind == "ExternalInput":
            if name != partition_name:
                in_names.append(name)
        elif alloc.kind == "ExternalOutput":
            out_names.append(name)
            out_avals.append(
                jax.core.ShapedArray(tuple(alloc.tensor_shape), mybir.dt.np(alloc.dtype))
            )
    n_params = len(in_names)
    n_outs = len(out_avals)
    all_names = in_names + out_names + ([partition_name] if partition_name else [])
    donate = tuple(range(n_params, n_params + n_outs))

    def _body(*args):
        operands = list(args)
        if partition_name is not None:
            operands.append(bass2jax.partition_id_tensor())
        return tuple(
            bass2jax._bass_exec_p.bind(
                *operands,
                out_avals=tuple(out_avals),
                in_names=tuple(all_names),
                out_names=tuple(out_names),
                lowering_input_output_aliases=(),
                sim_require_finite=True,
                sim_require_nnan=True,
                nc=nc,
            )
        )

    devices = jax.devices()[:N_CORES]
    mesh = Mesh(np.asarray(devices), ("core",))
    spec = NamedSharding(mesh, PartitionSpec("core"))
    sharded = jax.jit(
        shard_map(
            _body,
            mesh=mesh,
            in_specs=(PartitionSpec("core"),) * (n_params + n_outs),
            out_specs=(PartitionSpec("core"),) * n_outs,
            check_rep=False,
        ),
        donate_argnums=donate,
        keep_unused=True,
    )

    def run(u_cores, mats_cores):
        per_core = {
            "u": np.concatenate(u_cores, axis=0),
            "mats": np.concatenate(mats_cores, axis=0),
        }
        xs = [jax.device_put(per_core[nm], spec) for nm in in_names]
        zs = [
            jax.device_put(
                np.zeros((N_CORES * a.shape[0], *a.shape[1:]), a.dtype), spec
            )
            for a in out_avals
        ]
        outs = sharded(*xs, *zs)
        out = np.asarray(outs[out_names.index("out")])
        return out.reshape(N_CORES, N_SLAB, S, S)

    _CACHE["runner"] = run
    return run


def kernel(u, alpha_base, beta_base, alpha_spatial, beta_spatial, channel_coupling):
    from concourse._compat import axon_active

    u = np.ascontiguousarray(np.asarray(u, dtype=np.float32))
    mats_full = _host_mats(
        np.asarray(alpha_base, dtype=np.float32),
        np.asarray(beta_base, dtype=np.float32),
        np.asarray(alpha_spatial, dtype=np.float32),
        np.asarray(beta_spatial, dtype=np.float32),
        np.asarray(channel_coupling, dtype=np.float32),
    )
    u_cores, mats_cores, idxs = [], [], []
    for k in range(N_CORES):
        b_idx, c_idx = _core_slab_indices(k)
        idxs.append((b_idx, c_idx))
        u_cores.append(np.ascontiguousarray(u[b_idx, c_idx]))
        (c32, _), (c16, _) = ASSIGN[k]
        mats_cores.append(np.stack([mats_full[c32], mats_full[c16]]))

    if axon_active():
        res = _axon_runner()(u_cores, mats_cores)
    else:
        # Native path (/dev/neuron* present): run via NRT on cores 0-7.
        from concourse.bass_utils import run_bass_kernel_spmd

        nc = _CACHE.setdefault("nc", build_module())
        in_maps = [
            {"u": u_cores[k], "mats": mats_cores[k]} for k in range(N_CORES)
        ]
        rr = run_bass_kernel_spmd(nc, in_maps, core_ids=list(range(N_CORES)))
        res = np.stack([r["out"] for r in rr.results])

    out = np.empty((B, C, S, S), dtype=np.float32)
    for k in range(N_CORES):
        b_idx, c_idx = idxs[k]
        out[b_idx, c_idx] = res[k]
    return out
